# revision 48
# baseline (speedup 1.0000x reference)
"""Trainium2 Bass kernel for nn_MinibatchDiscrimination.

Reference math (f32):
    M = einsum('bi,ijk->bjk', x, T)                     # [512, 64, 16]
    L1[i,j,o] = sum_k |M[i,o,k] - M[j,o,k]|             # [512, 512, 64]
    c = exp(-L1) * (1 - eye)                            # mask self-pairs
    o_b = 0.5 * c.mean(axis=1)                          # [512, 64]
    out = concat([x, o_b], axis=1)                      # [512, 320]

Sharding: the i-index of the pairwise computation is split across 8 cores
(64 rows each). SPMD-uniform: each core receives x ROTATED by -64*c rows so
its slab lands at pair-columns j'=0..63; only input DATA differs per core.

Symmetry: c[i,j]=c[j,i]; each row il only processes the 256-wide window
j' in [il+1, il+256]. Row-sums land in A (Exp accum_out); the partner side
is scattered via column-partials C (window columns 1..255; the d=256 pair is
covered by both rows' A). Host combines A + C from all cores.

Abs via ReLU + linear correction (the fused TensorScalar abs_max dual-op is
not a valid TRN2 ISA op, but (subtract, max) is):
    |d| = 2*max(d,0) - d,  and  sum_k d_k = S[j,o] - S[i,o],
    S[j,o] = sum_k M[j,o,k]  (computed once per core)
so  L1[i,j,o] = 2*sum_k ReLU(d_k) - S[j,o] + S[i,o].
Per row, PSUM accumulates 2*ReLU contractions (indicator entries = 2.0) and
one -S[j] correction matmul (-identity stationary over a window of S^T);
the +S[i] term rides the Exp bias (per-partition scalar).

Engine split per row (u = pairs of k, partition layout p=(s,o), k=2u+s):
  - u=0..5 on DVE: TensorScalar (subtract, max) bf16 -> DVE 4x perf mode;
  - u=6 on Pool: same op, fp8e4 out;
  - u=7 on ACT: Relu activation (bias = -M^T[:,il]), fp8e4 out;
  - PE: 1 correction + 6 bf16 indicator matmuls + ONE fp8 DoubleRow matmul
    (contracts u=6,7 together at half cost) per row.
Rows are processed in PAIRS sharing one [128, 256] PSUM tile (row parity ->
partition half): ONE Exp (scale=-1, bias=-S[i], accum_out->A2[:,t]) per pair
and ONE Pool add scatters E[:, :255] into C2 (half-1 tile columns are
global-1; the host shifts them back).

bf16/fp8 are safe: L1 is O(100) so c=exp(-L1) ~ 1e-36; percent-level L1
error moves c by orders of magnitude below the output scale (max|out| ~ 4.8
from the x passthrough, which stays exact).
"""

import numpy as np
from contextlib import ExitStack

import concourse.tile as tile
from concourse import bacc, mybir
from concourse.bass_utils import run_bass_kernel_spmd

F32 = mybir.dt.float32
BF16 = mybir.dt.bfloat16
F8E4 = mybir.dt.float8e4
ALU = mybir.AluOpType
AF = mybir.ActivationFunctionType

B = 512          # batch
INF = 256        # in_features
OUTF = 64        # out_features
KD = 16          # kernel dims
N_CORES = 8
SLAB = B // N_CORES          # 64 rows of i per core
NPAIR = SLAB // 2            # 32 row pairs per core
NU = KD // 2                 # 8 u-chunks (pairs of k)
NDVE = 6                     # u 0..5 on DVE; u 6 Pool; u 7 ACT
W = 256                      # symmetric window width
MTW = 320                    # produced MT4 width (max col ever read: 319)
OSCALE = 0.5 / B             # exact power of two (2^-10)


def _build_nc():
    nc = bacc.Bacc("TRN2", target_bir_lowering=False, debug=False)

    # x^T host-rolled/transposed/trimmed per core, bf16: [i, j'] with
    # j' < MTW+1 (the row-parity-shifted S build reads one extra column)
    xt_d = nc.dram_tensor("xT", [INF, MTW + 1], BF16, kind="ExternalInput").ap()
    # T host-permuted to [i, (u, s, o)]: column u*128 + s*64 + o = T[i, o, 2u+s]
    t_d = nc.dram_tensor("Tp", [INF, OUTF * KD], BF16, kind="ExternalInput").ap()
    # bf16 const blob: cols 0:64 = 2x indicator (ReLU doubling folded in);
    # cols 64:128 rows 64:128 = -I(64) (odd-row -S[j] stationary, base-64);
    # cols 192:320 rows 0:64 = 0.5x s-duplicating indicator (SROW8 build)
    cb_d = nc.dram_tensor("cblob", [128, 5 * OUTF], BF16, kind="ExternalInput").ap()
    # fp8 indicators doubled along the DoubleRow dim: cols 0:128 = 2x
    # (ReLU chunks), 128:256 = (-1, 0) for the even-row -S[j] via SROW8
    i8_d = nc.dram_tensor("ind8", [128, 4 * OUTF], F8E4, kind="ExternalInput").ap()
    a_d = nc.dram_tensor("outa", [128, NPAIR], F32, kind="ExternalOutput").ap()
    c_d = nc.dram_tensor("outc", [128, MTW], BF16, kind="ExternalOutput").ap()

    with tile.TileContext(nc) as tc, ExitStack() as ctx:
        consts = ctx.enter_context(tc.tile_pool(name="consts", bufs=1))
        work = ctx.enter_context(tc.tile_pool(name="work", bufs=1))
        epool = ctx.enter_context(tc.tile_pool(name="epool", bufs=8))

        # ---- constants (one blob DMA; slices are views) ----
        cblob = consts.tile([128, 5 * OUTF], BF16, tag="cb", name="cblob")
        nc.gpsimd.dma_start(cblob, cb_d)
        ind2 = cblob[:, 0:OUTF]
        indneg64h = cblob[OUTF:128, OUTF:2 * OUTF]
        dupind = cblob[0:OUTF, 3 * OUTF:5 * OUTF]
        ind8 = consts.tile([128, 4 * OUTF], F8E4, tag="i8", name="ind8")
        nc.gpsimd.dma_start(ind8, i8_d)

        MT4 = consts.tile([128, NU * MTW], BF16, tag="mt4", name="MT4")
        COLS = consts.tile([128, NU * SLAB], F32, tag="cols", name="COLS")
        # negated columns for the ACT Relu bias (only u=7 is read, but a
        # single bulk negate is cheaper than 8 small ones)
        COLSN = consts.tile([128, NU * SLAB], F32, tag="colsn", name="COLSN")
        # SROWPAIR[(rr, o), j] = S[o, j + rr]: both rows of a pair read
        # their -S[j] correction window from one 128-partition tile
        SROWPAIR = consts.tile([128, MTW], BF16, tag="srp", name="SROWPAIR")
        SROW8 = consts.tile([128, MTW], F8E4, tag="srow8", name="SROW8")
        SB2 = consts.tile([128, NPAIR], F32, tag="sb2", name="SB2")
        # Static D/d8 tiles (two alternating per-pair sets) instead of a
        # rotating pool: the pool-slot recycle emits a same-engine WAW
        # semaphore wait per op, which costs a 70ns DVE SEQ slot and paces
        # the whole main loop; with static tiles only the cross-engine WAR
        # (PE consumed the previous use) remains, and it rides inline.
        dstat = [work.tile([128, W], BF16, tag=f"Ds{i}", name=f"Dstat_{i}")
                 for i in range(36)]
        d8stat = [work.tile([128, 2 * W], F8E4, tag=f"D8s{i}", name=f"D8stat_{i}")
                  for i in range(4)]
        A2 = work.tile([128, NPAIR], F32, tag="A2", name="A2")
        C2 = work.tile([128, MTW], BF16, tag="C2", name="C2")
        nc.gpsimd.memset(C2, 0.0)

        # ---- production ----
        with tc.tile_pool(name="prod", bufs=1) as prod, \
             tc.tile_pool(name="ps_prod", bufs=2, space="PSUM") as ps_prod, \
             tc.tile_pool(name="ps_s", bufs=1, space="PSUM") as ps_s:
            # spread the input DMAs across DGE queues (SP / ACT / DVE) so
            # their ~625ns descriptor generations overlap instead of
            # serializing on one queue
            t_sb = []
            xT = []
            dma_eng = [nc.sync, nc.scalar, nc.scalar, nc.sync]
            for ic in range(2):
                tt_ = prod.tile([128, OUTF * KD], BF16,
                                tag=f"tsb{ic}", name=f"t_sb{ic}")
                dma_eng[ic].dma_start(tt_, t_d[ic * 128:(ic + 1) * 128, :])
                t_sb.append(tt_)
                xt2_ = prod.tile([128, MTW + 1], BF16,
                                 tag=f"xT{ic}", name=f"xT{ic}")
                dma_eng[2 + ic].dma_start(xt2_, xt_d[ic * 128:(ic + 1) * 128, :])
                xT.append(xt2_)

            # -Tsum[i, o] = -sum_k T[i, o, k] via a strided halving tree.
            # Runs on Pool (idle during production; DVE's span is the wall).
            negts = []
            posts = []
            for ic in range(2):
                s1 = prod.tile([128, 512], BF16, tag=f"s1{ic}", name=f"s1_{ic}")
                nc.vector.tensor_tensor(
                    s1, t_sb[ic][:, 0:512], t_sb[ic][:, 512:1024], ALU.add)
                s2 = prod.tile([128, 256], BF16, tag=f"s2{ic}", name=f"s2_{ic}")
                nc.vector.tensor_tensor(s2, s1[:, 0:256], s1[:, 256:512], ALU.add)
                s3 = prod.tile([128, 128], BF16, tag=f"s3{ic}", name=f"s3_{ic}")
                nc.vector.tensor_tensor(s3, s2[:, 0:128], s2[:, 128:256], ALU.add)
                s4 = prod.tile([128, OUTF], BF16, tag=f"s4{ic}", name=f"s4_{ic}")
                nc.vector.tensor_scalar(
                    s4, s3[:, 0:OUTF], -1.0, None, ALU.mult)
                s5 = prod.tile([128, OUTF], BF16, tag=f"s5{ic}", name=f"s5_{ic}")
                nc.vector.tensor_tensor(s5, s4, s3[:, OUTF:128], ALU.subtract)
                negts.append(s5)
                s6 = prod.tile([128, OUTF], BF16, tag=f"s6{ic}", name=f"s6_{ic}")
                nc.vector.tensor_scalar(s6, s5, -1.0, None, ALU.mult)
                posts.append(s6)

            # SB2[(r,o), t] = -S[2t+r, o] (Exp bias), via strided xT columns
            sb2ps = ps_s.tile([128, NPAIR], F32, tag="sb2ps", name="sb2ps")
            for r in range(2):
                for ic in range(2):
                    nc.tensor.matmul(
                        sb2ps[r * OUTF:(r + 1) * OUTF, :],
                        negts[ic], xT[ic][:, r:SLAB:2],
                        start=(ic == 0), stop=(ic == 1))
            nc.vector.tensor_copy(SB2, sb2ps)

            # S[o, j+rr] = sum_i Tsum[i, o] x[j+rr, i], from the Tsum tree;
            # the rr=1 half reads a one-column-shifted xT window
            srowps = ps_s.tile([128, MTW], F32, tag="srowps", name="srowps")
            for rr in range(2):
                for ic in range(2):
                    nc.tensor.matmul(
                        srowps[rr * OUTF:(rr + 1) * OUTF, :],
                        posts[ic], xT[ic][:, rr:rr + MTW],
                        start=(ic == 0), stop=(ic == 1))
            nc.scalar.copy(SROWPAIR, srowps)

            # MT4[:, u*MTW:(u+1)*MTW][p=(s,o), j] = M[j, o, 2u+s]
            for u in range(NU):
                ps = ps_prod.tile([128, MTW], F32, tag="pst", name=f"ps_mt{u}")
                for ic in range(2):
                    lhs = t_sb[ic][:, u * 128:(u + 1) * 128]
                    nc.tensor.matmul(ps, lhs, xT[ic][:, 0:MTW],
                                     start=(ic == 0), stop=(ic == 1))
                # GPSIMD cannot touch PSUM: copies go on ACT/DVE
                mt_dst = MT4[:, u * MTW:(u + 1) * MTW]
                cdst = COLS[:, u * SLAB:(u + 1) * SLAB]
                if u % 2 == 0:
                    nc.scalar.copy(mt_dst, ps)
                    nc.vector.tensor_copy(cdst, ps[:, 0:SLAB])
                else:
                    nc.vector.tensor_copy(mt_dst, ps)
                    nc.scalar.copy(cdst, ps[:, 0:SLAB])
            nc.vector.tensor_scalar(COLSN, COLS, -1.0, None, ALU.mult)
            # SROW8[(s,o), j] = S[o, j]/2 duplicated across s halves (fp8),
            # for the even row's -S[j] DoubleRow correction
            sr8ps = ps_prod.tile([128, MTW], F32, tag="pst", name="sr8ps")
            nc.tensor.matmul(sr8ps, dupind, SROWPAIR[0:OUTF, :],
                             start=True, stop=True)
            nc.scalar.copy(SROW8, sr8ps)

        ind8v = ind8[:, 0:2 * OUTF].rearrange("p (two m) -> p two m", two=2)
        ind8n = ind8[:, 2 * OUTF:4 * OUTF].rearrange("p (two m) -> p two m", two=2)

        # ---- main loop over row PAIRS ----
        # DoubleRow matmuls may only write PSUM partition base 0, so ALL four
        # fp8 chunks (Pool u4,u5 + ACT u6,u7) go to the EVEN row; the odd row
        # is 8 bf16 DVE chunks. The L1 PSUM pool is allocated AFTER
        # production so it gets all 8 banks; the Exp/C epilogue is issued two
        # pairs late.
        ps_l1 = ctx.enter_context(tc.tile_pool(name="ps_l1", bufs=8, space="PSUM"))
        l1_tiles = [None] * NPAIR

        def epilogue(t):
            E = epool.tile([128, W], BF16, tag="E", name=f"E_{t}")
            nc.scalar.activation(
                E, l1_tiles[t], AF.Exp, scale=-1.0, bias=SB2[:, t:t + 1],
                accum_out=A2[:, t:t + 1])
            # column partials for both rows in one op; half-1 tile cols are
            # global-1 (host shifts back)
            cslice = C2[:, 2 * t + 1: 2 * t + W]
            nc.gpsimd.tensor_add(cslice, cslice, E[:, 0:W - 1])

        for t in range(NPAIR):
            L1 = ps_l1.tile([128, W], F32, tag="L1", name=f"L1_{t}")
            l1_tiles[t] = L1
            st = t % 3
            d_tiles = list(dstat[12 * st:12 * (st + 1)])
            if (t // 3) % 2:
                d_tiles.reverse()
            d_idx = 0
            il0, il1 = 2 * t, 2 * t + 1

            def win(u, il):
                return MT4[:, u * MTW + il + 1: u * MTW + il + 1 + W]

            def col(u, il):
                return COLS[:, u * SLAB + il: u * SLAB + il + 1]

            def ncol(u, il):
                return COLSN[:, u * SLAB + il: u * SLAB + il + 1]

            # fp8 chunk production (slow ACT/Pool ops) is issued at pair
            # START and consumed by the DoubleRows at pair END, giving the
            # producers a full pair of slack
            d8a = d8stat[2 * (t % 2)]
            d8b = d8stat[2 * (t % 2) + 1]
            nc.gpsimd.tensor_scalar(
                d8a[:, 0:W], win(4, il0), col(4, il0), 0.0, ALU.subtract, ALU.max)
            nc.gpsimd.tensor_scalar(
                d8a[:, W:2 * W], win(5, il0), col(5, il0), 0.0, ALU.subtract, ALU.max)
            nc.scalar.activation(
                d8b[:, 0:W], win(6, il0), AF.Relu, bias=ncol(6, il0))
            nc.scalar.activation(
                d8b[:, W:2 * W], win(7, il0), AF.Relu, bias=ncol(7, il0))

            # odd row: -S[j] (base-64 stationary) opens its half, 8 bf16
            # DVE chunks follow
            hi = L1[OUTF:128, :]
            nc.tensor.matmul(
                hi, indneg64h, SROWPAIR[OUTF:128, 2 * t + 1: 2 * t + 1 + W],
                start=True, stop=False)
            for u in range(NU):
                D = d_tiles[d_idx]
                d_idx += 1
                nc.vector.tensor_scalar(
                    D, win(u, il1), col(u, il1), 0.0, ALU.subtract, ALU.max)
                nc.tensor.matmul(hi, ind2, D, start=False, stop=(u == NU - 1))

            # even row: u0 opens the half, then 3 more bf16 chunks, the
            # -S[j] DoubleRow (stride-0 SROW8 window), and the two fp8
            # chunk DoubleRows
            lo = L1[0:OUTF, :]
            for u in range(4):
                D = d_tiles[d_idx]
                d_idx += 1
                nc.vector.tensor_scalar(
                    D, win(u, il0), col(u, il0), 0.0, ALU.subtract, ALU.max)
                nc.tensor.matmul(lo, ind2, D, start=(u == 0), stop=False)
            srw2 = SROW8[:, il0 + 1: il0 + 1 + W].unsqueeze(1) \
                .broadcast_to((128, 2, W))
            nc.tensor.matmul(lo, ind8n, srw2, start=False, stop=False,
                             perf_mode=mybir.MatmulPerfMode.DoubleRow)
            nc.tensor.matmul(
                lo, ind8v, d8a.rearrange("p (two n) -> p two n", two=2),
                start=False, stop=False,
                perf_mode=mybir.MatmulPerfMode.DoubleRow)
            nc.tensor.matmul(
                lo, ind8v, d8b.rearrange("p (two n) -> p two n", two=2),
                start=False, stop=True,
                perf_mode=mybir.MatmulPerfMode.DoubleRow)

            if t >= 2:
                epilogue(t - 2)
        epilogue(NPAIR - 2)
        epilogue(NPAIR - 1)

        nc.sync.dma_start(a_d, A2)
        nc.scalar.dma_start(c_d, C2)

    nc.compile()
    return nc


_NC = None


def _get_nc():
    global _NC
    if _NC is None:
        _NC = _build_nc()
    return _NC


def _host_inputs(x, T):
    import ml_dtypes
    ind = np.zeros((128, OUTF), np.float32)
    ind[np.arange(128), np.arange(128) % OUTF] = 1.0
    ind8 = np.concatenate([2.0 * ind, 2.0 * ind, -1.0 * ind, 0.0 * ind],
                          axis=1).astype(ml_dtypes.float8_e4m3)
    indneg = np.zeros((128, OUTF), np.float32)
    indneg[0:OUTF, 0:OUTF] = -np.eye(OUTF, dtype=np.float32)
    # dupind[o', (s,o)] = (o' == o): duplicates S across the two s halves
    negi64h = np.zeros((128, OUTF), np.float32)
    negi64h[OUTF:128, :] = -np.eye(OUTF, dtype=np.float32)
    dupind = np.zeros((128, 2 * OUTF), np.float32)
    dupind[0:OUTF, 0:OUTF] = 0.5 * np.eye(OUTF, dtype=np.float32)
    dupind[0:OUTF, OUTF:2 * OUTF] = 0.5 * np.eye(OUTF, dtype=np.float32)
    cblob = np.concatenate([2.0 * ind, negi64h, indneg, dupind], axis=1).astype(
        ml_dtypes.bfloat16)
    # [i, o, (u s)] -> [i, (u s o)]
    Tp = np.ascontiguousarray(
        T.reshape(INF, OUTF, NU, 2).transpose(0, 2, 3, 1).reshape(INF, OUTF * KD)
    ).astype(ml_dtypes.bfloat16)
    in_maps = []
    for c in range(N_CORES):
        xr = np.roll(x, -c * SLAB, axis=0)
        xrT = np.ascontiguousarray(xr[0:MTW + 1, :].T).astype(ml_dtypes.bfloat16)
        in_maps.append({"xT": xrT, "Tp": Tp, "cblob": cblob, "ind8": ind8})
    return in_maps


def _assemble(x, results):
    """Combine per-core row-sums and column-partials into the full output."""
    At = np.zeros((B, OUTF), np.float64)
    tt = np.arange(NPAIR)
    jj = np.arange(MTW)
    for c in range(N_CORES):
        a2 = np.asarray(results[c]["outa"], dtype=np.float64)  # [128, 32]
        c2 = np.asarray(results[c]["outc"], dtype=np.float64)  # [128, 320]
        for r in range(2):
            At[c * SLAB + 2 * tt + r, :] += a2[r * OUTF:(r + 1) * OUTF, :].T
        np.add.at(At, (jj + c * SLAB) % B, c2[0:OUTF, :].T)
        np.add.at(At, (jj + 1 + c * SLAB) % B, c2[OUTF:128, :].T)
    o_b = (At * OSCALE).astype(np.float32)
    return np.concatenate([x, o_b], axis=1)


def _run(x, T, trace=False):
    x = np.ascontiguousarray(np.asarray(x, dtype=np.float32))
    T = np.ascontiguousarray(np.asarray(T, dtype=np.float32))
    assert x.shape == (B, INF) and T.shape == (INF, OUTF, KD)
    nc = _get_nc()
    in_maps = _host_inputs(x, T)
    res = run_bass_kernel_spmd(nc, in_maps, list(range(N_CORES)), trace=trace)
    return _assemble(x, res.results), res


def kernel(x, T):
    out, _ = _run(x, T, trace=False)
    return out


def kernel_profiled(x, T):
    out, res = _run(x, T, trace=True)
    return out, res


# revision 49
# speedup vs baseline: 1.0033x; 1.0033x over previous
"""Trainium2 Bass kernel for nn_MinibatchDiscrimination.

Reference math (f32):
    M = einsum('bi,ijk->bjk', x, T)                     # [512, 64, 16]
    L1[i,j,o] = sum_k |M[i,o,k] - M[j,o,k]|             # [512, 512, 64]
    c = exp(-L1) * (1 - eye)                            # mask self-pairs
    o_b = 0.5 * c.mean(axis=1)                          # [512, 64]
    out = concat([x, o_b], axis=1)                      # [512, 320]

Sharding: the i-index of the pairwise computation is split across 8 cores
(64 rows each). SPMD-uniform: each core receives x ROTATED by -64*c rows so
its slab lands at pair-columns j'=0..63; only input DATA differs per core.

Symmetry: c[i,j]=c[j,i]; each row il only processes the 256-wide window
j' in [il+1, il+256]. Row-sums land in A (Exp accum_out); the partner side
is scattered via column-partials C (window columns 1..255; the d=256 pair is
covered by both rows' A). Host combines A + C from all cores.

Abs via ReLU + linear correction (the fused TensorScalar abs_max dual-op is
not a valid TRN2 ISA op, but (subtract, max) is):
    |d| = 2*max(d,0) - d,  and  sum_k d_k = S[j,o] - S[i,o],
    S[j,o] = sum_k M[j,o,k]  (computed once per core)
so  L1[i,j,o] = 2*sum_k ReLU(d_k) - S[j,o] + S[i,o].
Per row, PSUM accumulates 2*ReLU contractions (indicator entries = 2.0) and
one -S[j] correction matmul (-identity stationary over a window of S^T);
the +S[i] term rides the Exp bias (per-partition scalar).

Engine split per row (u = pairs of k, partition layout p=(s,o), k=2u+s):
  - u=0..5 on DVE: TensorScalar (subtract, max) bf16 -> DVE 4x perf mode;
  - u=6 on Pool: same op, fp8e4 out;
  - u=7 on ACT: Relu activation (bias = -M^T[:,il]), fp8e4 out;
  - PE: 1 correction + 6 bf16 indicator matmuls + ONE fp8 DoubleRow matmul
    (contracts u=6,7 together at half cost) per row.
Rows are processed in PAIRS sharing one [128, 256] PSUM tile (row parity ->
partition half): ONE Exp (scale=-1, bias=-S[i], accum_out->A2[:,t]) per pair
and ONE Pool add scatters E[:, :255] into C2 (half-1 tile columns are
global-1; the host shifts them back).

bf16/fp8 are safe: L1 is O(100) so c=exp(-L1) ~ 1e-36; percent-level L1
error moves c by orders of magnitude below the output scale (max|out| ~ 4.8
from the x passthrough, which stays exact).
"""

import numpy as np
from contextlib import ExitStack

import concourse.tile as tile
from concourse import bacc, mybir
from concourse.bass_utils import run_bass_kernel_spmd

F32 = mybir.dt.float32
BF16 = mybir.dt.bfloat16
F8E4 = mybir.dt.float8e4
ALU = mybir.AluOpType
AF = mybir.ActivationFunctionType

B = 512          # batch
INF = 256        # in_features
OUTF = 64        # out_features
KD = 16          # kernel dims
N_CORES = 8
SLAB = B // N_CORES          # 64 rows of i per core
NPAIR = SLAB // 2            # 32 row pairs per core
NU = KD // 2                 # 8 u-chunks (pairs of k)
NDVE = 6                     # u 0..5 on DVE; u 6 Pool; u 7 ACT
W = 256                      # symmetric window width
MTW = 320                    # produced MT4 width (max col ever read: 319)
OSCALE = 0.5 / B             # exact power of two (2^-10)


def _build_nc():
    nc = bacc.Bacc("TRN2", target_bir_lowering=False, debug=False)

    # x^T host-rolled/transposed/trimmed per core, bf16: [i, j'] with
    # j' < MTW+1 (the row-parity-shifted S build reads one extra column)
    xt_d = nc.dram_tensor("xT", [INF, MTW + 1], BF16, kind="ExternalInput").ap()
    # T host-permuted to [i, (u, s, o)]: column u*128 + s*64 + o = T[i, o, 2u+s]
    t_d = nc.dram_tensor("Tp", [INF, OUTF * KD], BF16, kind="ExternalInput").ap()
    # bf16 const blob: cols 0:64 = 2x indicator (ReLU doubling folded in);
    # cols 64:128 rows 64:128 = -I(64) (odd-row -S[j] stationary, base-64);
    # cols 192:320 rows 0:64 = 0.5x s-duplicating indicator (SROW8 build)
    cb_d = nc.dram_tensor("cblob", [128, 5 * OUTF], BF16, kind="ExternalInput").ap()
    # fp8 indicators doubled along the DoubleRow dim: cols 0:128 = 2x
    # (ReLU chunks), 128:256 = (-1, 0) for the even-row -S[j] via SROW8
    i8_d = nc.dram_tensor("ind8", [128, 4 * OUTF], F8E4, kind="ExternalInput").ap()
    a_d = nc.dram_tensor("outa", [128, NPAIR], F32, kind="ExternalOutput").ap()
    c_d = nc.dram_tensor("outc", [128, MTW], BF16, kind="ExternalOutput").ap()

    with tile.TileContext(nc) as tc, ExitStack() as ctx:
        consts = ctx.enter_context(tc.tile_pool(name="consts", bufs=1))
        work = ctx.enter_context(tc.tile_pool(name="work", bufs=1))
        epool = ctx.enter_context(tc.tile_pool(name="epool", bufs=8))

        # ---- constants (one blob DMA; slices are views) ----
        cblob = consts.tile([128, 5 * OUTF], BF16, tag="cb", name="cblob")
        nc.gpsimd.dma_start(cblob, cb_d)
        ind2 = cblob[:, 0:OUTF]
        indneg64h = cblob[OUTF:128, OUTF:2 * OUTF]
        dupind = cblob[0:OUTF, 3 * OUTF:5 * OUTF]
        ind8 = consts.tile([128, 4 * OUTF], F8E4, tag="i8", name="ind8")
        nc.gpsimd.dma_start(ind8, i8_d)

        MT4 = consts.tile([128, NU * MTW], BF16, tag="mt4", name="MT4")
        COLS = consts.tile([128, NU * SLAB], F32, tag="cols", name="COLS")
        # negated columns for the ACT Relu bias (only u=7 is read, but a
        # single bulk negate is cheaper than 8 small ones)
        COLSN = consts.tile([128, NU * SLAB], F32, tag="colsn", name="COLSN")
        # SROWPAIR[(rr, o), j] = S[o, j + rr]: both rows of a pair read
        # their -S[j] correction window from one 128-partition tile
        SROWPAIR = consts.tile([128, MTW], BF16, tag="srp", name="SROWPAIR")
        SROW8 = consts.tile([128, MTW], F8E4, tag="srow8", name="SROW8")
        SB2 = consts.tile([128, NPAIR], F32, tag="sb2", name="SB2")
        # Static D/d8 tiles (two alternating per-pair sets) instead of a
        # rotating pool: the pool-slot recycle emits a same-engine WAW
        # semaphore wait per op, which costs a 70ns DVE SEQ slot and paces
        # the whole main loop; with static tiles only the cross-engine WAR
        # (PE consumed the previous use) remains, and it rides inline.
        dstat = [work.tile([128, W], BF16, tag=f"Ds{i}", name=f"Dstat_{i}")
                 for i in range(36)]
        d8stat = [work.tile([128, 2 * W], F8E4, tag=f"D8s{i}", name=f"D8stat_{i}")
                  for i in range(4)]
        A2 = work.tile([128, NPAIR], F32, tag="A2", name="A2")
        C2 = work.tile([128, MTW], BF16, tag="C2", name="C2")
        nc.gpsimd.memset(C2, 0.0)

        # ---- production ----
        with tc.tile_pool(name="prod", bufs=1) as prod, \
             tc.tile_pool(name="ps_prod", bufs=2, space="PSUM") as ps_prod, \
             tc.tile_pool(name="ps_warm", bufs=1, space="PSUM") as ps_warm, \
             tc.tile_pool(name="ps_s", bufs=1, space="PSUM") as ps_s:
            # PE p-state warm-up: the ramp to 2.4GHz needs ~3us of
            # CONTINUOUS execution, but PE would otherwise idle through the
            # input DMAs and then run the whole production chain at 1.2GHz.
            # Dependency-free dummy matmuls on a zeroed tile start the ramp
            # clock immediately.
            warm = prod.tile([128, MTW], BF16, tag="warm", name="warm")
            nc.vector.memset(warm, 0.0)
            wps = ps_warm.tile([OUTF, MTW], F32, tag="wps", name="wps")
            for wi in range(10):
                nc.tensor.matmul(wps, warm[:, 0:OUTF], warm,
                                 start=True, stop=True)
            # spread the input DMAs across DGE queues (SP / ACT / DVE) so
            # their ~625ns descriptor generations overlap instead of
            # serializing on one queue
            t_sb = []
            xT = []
            dma_eng = [nc.sync, nc.scalar, nc.scalar, nc.sync]
            for ic in range(2):
                tt_ = prod.tile([128, OUTF * KD], BF16,
                                tag=f"tsb{ic}", name=f"t_sb{ic}")
                dma_eng[ic].dma_start(tt_, t_d[ic * 128:(ic + 1) * 128, :])
                t_sb.append(tt_)
                xt2_ = prod.tile([128, MTW + 1], BF16,
                                 tag=f"xT{ic}", name=f"xT{ic}")
                dma_eng[2 + ic].dma_start(xt2_, xt_d[ic * 128:(ic + 1) * 128, :])
                xT.append(xt2_)

            # -Tsum[i, o] = -sum_k T[i, o, k] via a strided halving tree.
            # Runs on Pool (idle during production; DVE's span is the wall).
            negts = []
            posts = []
            for ic in range(2):
                s1 = prod.tile([128, 512], BF16, tag=f"s1{ic}", name=f"s1_{ic}")
                nc.vector.tensor_tensor(
                    s1, t_sb[ic][:, 0:512], t_sb[ic][:, 512:1024], ALU.add)
                s2 = prod.tile([128, 256], BF16, tag=f"s2{ic}", name=f"s2_{ic}")
                nc.vector.tensor_tensor(s2, s1[:, 0:256], s1[:, 256:512], ALU.add)
                s3 = prod.tile([128, 128], BF16, tag=f"s3{ic}", name=f"s3_{ic}")
                nc.vector.tensor_tensor(s3, s2[:, 0:128], s2[:, 128:256], ALU.add)
                s4 = prod.tile([128, OUTF], BF16, tag=f"s4{ic}", name=f"s4_{ic}")
                nc.vector.tensor_scalar(
                    s4, s3[:, 0:OUTF], -1.0, None, ALU.mult)
                s5 = prod.tile([128, OUTF], BF16, tag=f"s5{ic}", name=f"s5_{ic}")
                nc.vector.tensor_tensor(s5, s4, s3[:, OUTF:128], ALU.subtract)
                negts.append(s5)
                s6 = prod.tile([128, OUTF], BF16, tag=f"s6{ic}", name=f"s6_{ic}")
                nc.vector.tensor_scalar(s6, s5, -1.0, None, ALU.mult)
                posts.append(s6)

            # SB2[(r,o), t] = -S[2t+r, o] (Exp bias), via strided xT columns
            sb2ps = ps_s.tile([128, NPAIR], F32, tag="sb2ps", name="sb2ps")
            for r in range(2):
                for ic in range(2):
                    nc.tensor.matmul(
                        sb2ps[r * OUTF:(r + 1) * OUTF, :],
                        negts[ic], xT[ic][:, r:SLAB:2],
                        start=(ic == 0), stop=(ic == 1))
            nc.vector.tensor_copy(SB2, sb2ps)

            # S[o, j+rr] = sum_i Tsum[i, o] x[j+rr, i], from the Tsum tree;
            # the rr=1 half reads a one-column-shifted xT window
            srowps = ps_s.tile([128, MTW], F32, tag="srowps", name="srowps")
            for rr in range(2):
                for ic in range(2):
                    nc.tensor.matmul(
                        srowps[rr * OUTF:(rr + 1) * OUTF, :],
                        posts[ic], xT[ic][:, rr:rr + MTW],
                        start=(ic == 0), stop=(ic == 1))
            nc.scalar.copy(SROWPAIR, srowps)

            # MT4[:, u*MTW:(u+1)*MTW][p=(s,o), j] = M[j, o, 2u+s]
            for u in range(NU):
                ps = ps_prod.tile([128, MTW], F32, tag="pst", name=f"ps_mt{u}")
                for ic in range(2):
                    lhs = t_sb[ic][:, u * 128:(u + 1) * 128]
                    nc.tensor.matmul(ps, lhs, xT[ic][:, 0:MTW],
                                     start=(ic == 0), stop=(ic == 1))
                # GPSIMD cannot touch PSUM: copies go on ACT/DVE
                mt_dst = MT4[:, u * MTW:(u + 1) * MTW]
                cdst = COLS[:, u * SLAB:(u + 1) * SLAB]
                if u % 2 == 0:
                    nc.scalar.copy(mt_dst, ps)
                    nc.vector.tensor_copy(cdst, ps[:, 0:SLAB])
                else:
                    nc.vector.tensor_copy(mt_dst, ps)
                    nc.scalar.copy(cdst, ps[:, 0:SLAB])
            nc.vector.tensor_scalar(COLSN, COLS, -1.0, None, ALU.mult)
            # SROW8[(s,o), j] = S[o, j]/2 duplicated across s halves (fp8),
            # for the even row's -S[j] DoubleRow correction
            sr8ps = ps_prod.tile([128, MTW], F32, tag="pst", name="sr8ps")
            nc.tensor.matmul(sr8ps, dupind, SROWPAIR[0:OUTF, :],
                             start=True, stop=True)
            nc.scalar.copy(SROW8, sr8ps)

        ind8v = ind8[:, 0:2 * OUTF].rearrange("p (two m) -> p two m", two=2)
        ind8n = ind8[:, 2 * OUTF:4 * OUTF].rearrange("p (two m) -> p two m", two=2)

        # ---- main loop over row PAIRS ----
        # DoubleRow matmuls may only write PSUM partition base 0, so ALL four
        # fp8 chunks (Pool u4,u5 + ACT u6,u7) go to the EVEN row; the odd row
        # is 8 bf16 DVE chunks. The L1 PSUM pool is allocated AFTER
        # production so it gets all 8 banks; the Exp/C epilogue is issued two
        # pairs late.
        ps_l1 = ctx.enter_context(tc.tile_pool(name="ps_l1", bufs=8, space="PSUM"))
        l1_tiles = [None] * NPAIR

        def epilogue(t):
            E = epool.tile([128, W], BF16, tag="E", name=f"E_{t}")
            nc.scalar.activation(
                E, l1_tiles[t], AF.Exp, scale=-1.0, bias=SB2[:, t:t + 1],
                accum_out=A2[:, t:t + 1])
            # column partials for both rows in one op; half-1 tile cols are
            # global-1 (host shifts back)
            cslice = C2[:, 2 * t + 1: 2 * t + W]
            nc.gpsimd.tensor_add(cslice, cslice, E[:, 0:W - 1])

        for t in range(NPAIR):
            L1 = ps_l1.tile([128, W], F32, tag="L1", name=f"L1_{t}")
            l1_tiles[t] = L1
            st = t % 3
            d_tiles = list(dstat[12 * st:12 * (st + 1)])
            if (t // 3) % 2:
                d_tiles.reverse()
            d_idx = 0
            il0, il1 = 2 * t, 2 * t + 1

            def win(u, il):
                return MT4[:, u * MTW + il + 1: u * MTW + il + 1 + W]

            def col(u, il):
                return COLS[:, u * SLAB + il: u * SLAB + il + 1]

            def ncol(u, il):
                return COLSN[:, u * SLAB + il: u * SLAB + il + 1]

            # fp8 chunk production (slow ACT/Pool ops) is issued at pair
            # START and consumed by the DoubleRows at pair END, giving the
            # producers a full pair of slack
            d8a = d8stat[2 * (t % 2)]
            d8b = d8stat[2 * (t % 2) + 1]
            nc.gpsimd.tensor_scalar(
                d8a[:, 0:W], win(4, il0), col(4, il0), 0.0, ALU.subtract, ALU.max)
            nc.gpsimd.tensor_scalar(
                d8a[:, W:2 * W], win(5, il0), col(5, il0), 0.0, ALU.subtract, ALU.max)
            nc.scalar.activation(
                d8b[:, 0:W], win(6, il0), AF.Relu, bias=ncol(6, il0))
            nc.scalar.activation(
                d8b[:, W:2 * W], win(7, il0), AF.Relu, bias=ncol(7, il0))

            # odd row: -S[j] (base-64 stationary) opens its half, 8 bf16
            # DVE chunks follow
            hi = L1[OUTF:128, :]
            nc.tensor.matmul(
                hi, indneg64h, SROWPAIR[OUTF:128, 2 * t + 1: 2 * t + 1 + W],
                start=True, stop=False)
            for u in range(NU):
                D = d_tiles[d_idx]
                d_idx += 1
                nc.vector.tensor_scalar(
                    D, win(u, il1), col(u, il1), 0.0, ALU.subtract, ALU.max)
                nc.tensor.matmul(hi, ind2, D, start=False, stop=(u == NU - 1))

            # even row: u0 opens the half, then 3 more bf16 chunks, the
            # -S[j] DoubleRow (stride-0 SROW8 window), and the two fp8
            # chunk DoubleRows
            lo = L1[0:OUTF, :]
            for u in range(4):
                D = d_tiles[d_idx]
                d_idx += 1
                nc.vector.tensor_scalar(
                    D, win(u, il0), col(u, il0), 0.0, ALU.subtract, ALU.max)
                nc.tensor.matmul(lo, ind2, D, start=(u == 0), stop=False)
            srw2 = SROW8[:, il0 + 1: il0 + 1 + W].unsqueeze(1) \
                .broadcast_to((128, 2, W))
            nc.tensor.matmul(lo, ind8n, srw2, start=False, stop=False,
                             perf_mode=mybir.MatmulPerfMode.DoubleRow)
            nc.tensor.matmul(
                lo, ind8v, d8a.rearrange("p (two n) -> p two n", two=2),
                start=False, stop=False,
                perf_mode=mybir.MatmulPerfMode.DoubleRow)
            nc.tensor.matmul(
                lo, ind8v, d8b.rearrange("p (two n) -> p two n", two=2),
                start=False, stop=True,
                perf_mode=mybir.MatmulPerfMode.DoubleRow)

            if t >= 2:
                epilogue(t - 2)
        epilogue(NPAIR - 2)
        epilogue(NPAIR - 1)

        nc.sync.dma_start(a_d, A2)
        nc.scalar.dma_start(c_d, C2)

    nc.compile()
    return nc


_NC = None


def _get_nc():
    global _NC
    if _NC is None:
        _NC = _build_nc()
    return _NC


def _host_inputs(x, T):
    import ml_dtypes
    ind = np.zeros((128, OUTF), np.float32)
    ind[np.arange(128), np.arange(128) % OUTF] = 1.0
    ind8 = np.concatenate([2.0 * ind, 2.0 * ind, -1.0 * ind, 0.0 * ind],
                          axis=1).astype(ml_dtypes.float8_e4m3)
    indneg = np.zeros((128, OUTF), np.float32)
    indneg[0:OUTF, 0:OUTF] = -np.eye(OUTF, dtype=np.float32)
    # dupind[o', (s,o)] = (o' == o): duplicates S across the two s halves
    negi64h = np.zeros((128, OUTF), np.float32)
    negi64h[OUTF:128, :] = -np.eye(OUTF, dtype=np.float32)
    dupind = np.zeros((128, 2 * OUTF), np.float32)
    dupind[0:OUTF, 0:OUTF] = 0.5 * np.eye(OUTF, dtype=np.float32)
    dupind[0:OUTF, OUTF:2 * OUTF] = 0.5 * np.eye(OUTF, dtype=np.float32)
    cblob = np.concatenate([2.0 * ind, negi64h, indneg, dupind], axis=1).astype(
        ml_dtypes.bfloat16)
    # [i, o, (u s)] -> [i, (u s o)]
    Tp = np.ascontiguousarray(
        T.reshape(INF, OUTF, NU, 2).transpose(0, 2, 3, 1).reshape(INF, OUTF * KD)
    ).astype(ml_dtypes.bfloat16)
    in_maps = []
    for c in range(N_CORES):
        xr = np.roll(x, -c * SLAB, axis=0)
        xrT = np.ascontiguousarray(xr[0:MTW + 1, :].T).astype(ml_dtypes.bfloat16)
        in_maps.append({"xT": xrT, "Tp": Tp, "cblob": cblob, "ind8": ind8})
    return in_maps


def _assemble(x, results):
    """Combine per-core row-sums and column-partials into the full output."""
    At = np.zeros((B, OUTF), np.float64)
    tt = np.arange(NPAIR)
    jj = np.arange(MTW)
    for c in range(N_CORES):
        a2 = np.asarray(results[c]["outa"], dtype=np.float64)  # [128, 32]
        c2 = np.asarray(results[c]["outc"], dtype=np.float64)  # [128, 320]
        for r in range(2):
            At[c * SLAB + 2 * tt + r, :] += a2[r * OUTF:(r + 1) * OUTF, :].T
        np.add.at(At, (jj + c * SLAB) % B, c2[0:OUTF, :].T)
        np.add.at(At, (jj + 1 + c * SLAB) % B, c2[OUTF:128, :].T)
    o_b = (At * OSCALE).astype(np.float32)
    return np.concatenate([x, o_b], axis=1)


def _run(x, T, trace=False):
    x = np.ascontiguousarray(np.asarray(x, dtype=np.float32))
    T = np.ascontiguousarray(np.asarray(T, dtype=np.float32))
    assert x.shape == (B, INF) and T.shape == (INF, OUTF, KD)
    nc = _get_nc()
    in_maps = _host_inputs(x, T)
    res = run_bass_kernel_spmd(nc, in_maps, list(range(N_CORES)), trace=trace)
    return _assemble(x, res.results), res


def kernel(x, T):
    out, _ = _run(x, T, trace=False)
    return out


def kernel_profiled(x, T):
    out, res = _run(x, T, trace=True)
    return out, res
